# revision 4
# baseline (speedup 1.0000x reference)
"""Distributed causal MHA + RoPE kernel for 8 TRN2 NeuronCores (raw Bass), v2.

Reference (B=2, T=2048, D=1024, H=16, DH=64):
    qkv = x @ Wqkv -> per-head q,k,v -> RoPE(q,k)
    attn = softmax(causal(q k^T / 8)) @ v ;  out = concat_heads(attn) @ Wout

Sharding: 8 cores = 2 batches x 4 head-groups (4 heads each). Each core
emits a bf16 partial of the out-projection (its heads' rows of Wout);
the 4 partials per batch are summed on the host.

v2 structure (vs v1):
 - phases interleaved per 512-token group t: A(t) qk-proj+rope, B(t) V,
   C(t,hp=0/1) attention for query block t, D(t-1) out-projection --
   causal attention for query block t only needs K/V tokens <= 512(t+1).
 - phase C software pipeline: PE stream runs [exp-gate, S(kj+2) pair,
   PV(kj) pair] per kj; S^T tiles double-buffered across 4 PSUM banks
   (P4, kj parity x head-in-pair), exp fused over both heads in one ACT
   instruction reading 2 adjacent banks; causal suffix-trim shrinks
   S/exp/PV to the unmasked column range; triangular mask multiply only
   on the [128,128] diagonal block (DVE).
 - rope in 6 wide DVE ops per tile via sign-folded sin tables and
   partition-shifted writes (out = qk*cos + swap32(qk*sinF)).
 - softmax denominators from a ones-column in V' (PV M=65); the
   normalize 1/d column-broadcast goes through a DRAM round trip whose
   dependent multiplies are emitted one sub-phase later, keeping the
   DVE stream from blocking; PV accumulators double-buffered by hp.
 - phase D: K=128 head-pair matmuls, PSUM->bf16 copies split ACT/DVE,
   two row-blocks per output DMA; bf16 partials summed on host.
 - PE warmup matmuls during the input DMAs (clock-gate ramp).
"""

import numpy as np

B, T, D, H, DH = 2, 2048, 1024, 16, 64
HPC = 4            # heads per core
NG = 4             # head groups
TQ = 512
NQT = T // TQ      # 4
NKT = T // 128     # 16
KC = D // 128      # 8


def _build_nc(causal: bool):
    import concourse.bass as bass
    import concourse.mybir as mybir
    from contextlib import ExitStack

    dt = mybir.dt
    f32, bf16 = dt.float32, dt.bfloat16
    AF = mybir.ActivationFunctionType
    nc = bass.Bass()

    xT = nc.declare_dram_parameter("xT", [D, T], bf16, isOutput=False)
    wqk = nc.declare_dram_parameter("wqk", [D, 512], bf16, isOutput=False)
    wv = nc.declare_dram_parameter("wv", [D, 256], bf16, isOutput=False)
    wo = nc.declare_dram_parameter("wo", [256, D], bf16, isOutput=False)
    cos2 = nc.declare_dram_parameter("cos2", [128, T], bf16, isOutput=False)
    sinF = nc.declare_dram_parameter("sinF", [128, T], bf16, isOutput=False)
    dm1 = nc.declare_dram_parameter("dm1", [128, 128], bf16, isOutput=False)
    out = nc.declare_dram_parameter("out", [T, D], bf16, isOutput=True)
    rec_dram = nc.dram_tensor("rec_dram", [2, 2, TQ], f32)

    ctx = ExitStack()
    with ctx:
        sb = lambda name, shape, dtype: ctx.enter_context(
            nc.sbuf_tensor(name, shape, dtype))
        ps = lambda name, shape: ctx.enter_context(
            nc.psum_tensor(name, shape, f32))

        wqk_sb = sb("wqk_sb", [128, KC, 512], bf16)
        wv_sb = sb("wv_sb", [128, KC, 256], bf16)
        wo_sb = sb("wo_sb", [128, 2, D], bf16)       # head pairs stacked on K
        cos_sb = sb("cos_sb", [128, T], bf16)
        sin_sb = sb("sin_sb", [128, T], bf16)        # sign-folded
        dm_sb = sb("dm_sb", [128, 128], bf16)        # triangular k<=q
        xt_sb = sb("xt_sb", [128, KC, T], bf16)
        qk_sb = sb("qk_sb", [128, 4, TQ], bf16)      # A-copy staging (4-deep)
        tmp_sb = sb("tmp_sb", [128, 2, TQ], bf16)    # rope cos-prod / sin-prod
        qkr_sb = sb("qkr_sb", [128, 4, T], bf16)     # post-rope q01,q23,k01,k23
        vp_sb = sb("vp_sb", [128, NKT, HPC * 65], bf16)
        p_sb = sb("p_sb", [128, 4, 2, TQ], bf16)     # (kj%4 slot, hh)
        at2_sb = sb("at2_sb", [128, 2, T], bf16)     # o^T, head pairs on K
        rec_sb = sb("rec_sb", [1, 2, TQ], f32)
        rb_sb = sb("rb_sb", [64, 2, 2, TQ], f32)     # (buf, hh)
        ob_sb = sb("ob_sb", [128, 4, 2, 512], bf16)  # (tq%4, n)

        # PSUM: 8 banks total.  P4 = 4 adjacent banks: (parity, hh) halves
        # for S^T tiles -- adjacency lets one ACT exp cover 2 kj x 2 hh.
        # pO[2*buf+hh] 1 bank each for PV accum; phases A/B/D reuse these
        # banks (A/B: 4 banks of P4; D: pO[2], pO[3]).
        P4 = ps("P4", [128, 2048])
        pO = [ps(f"pO{i}", [128, 512]) for i in range(4)]

        sem_names = (["pe", "act", "dve", "bc", "out0", "out1"]
                     + [f"in{i}" for i in range(8)])
        sems = {n: ctx.enter_context(nc.semaphore(f"s_{n}")) for n in sem_names}
        block = ctx.enter_context(nc.Block())

        # ---------- schedule construction ----------
        sched = []  # (engine, fn)
        cnt = {n: 0 for n in sem_names}
        last_wait = {}

        def wait(eng, sem, val):
            if val <= 0:
                return
            key = (eng, sem)
            if last_wait.get(key, -1) >= val:
                return
            last_wait[key] = val
            sched.append((eng, lambda e, s=sems[sem], v=val: e.wait_ge(s, v)))

        def emit(eng, fn, inc=None, inc_by=1):
            if inc is None:
                sched.append((eng, fn))
            else:
                s = sems[inc]
                sched.append((eng, lambda e, f=fn, ss=s, ib=inc_by: f(e).then_inc(ss, ib)))
                cnt[inc] += inc_by

        def bankA(i):
            return P4.rearrange("p (b q) -> p b q", q=512)[:, i % 4, :]

        # ---- input DMAs (sync engine queue, ordered by first use) ----
        def dma_in(sem_i, dst, src):
            emit("sync", lambda e, d=dst, s=src: e.dma_start(out=d, in_=s),
                 inc=f"in{sem_i}", inc_by=16)

        wqk_r = wqk.rearrange("(c p) m -> p c m", p=128)
        dma_in(4, wqk_sb[:, :, 0:128], wqk_r[:, :, 0:128])
        dma_in(0, xt_sb[:, :, 0:TQ], xT.rearrange("(c p) t -> p c t", p=128)[:, :, 0:TQ])
        for m in range(1, 4):
            dma_in(4, wqk_sb[:, :, m * 128:(m + 1) * 128],
                   wqk_r[:, :, m * 128:(m + 1) * 128])
        dma_in(7, cos_sb[:], cos2[:])
        dma_in(7, sin_sb[:], sinF[:])
        dma_in(5, wv_sb[:], wv.rearrange("(c p) m -> p c m", p=128))
        dma_in(1, xt_sb[:, :, TQ:2 * TQ],
               xT.rearrange("(c p) t -> p c t", p=128)[:, :, TQ:2 * TQ])
        dma_in(7, dm_sb[:], dm1[:])
        for t in range(2, 4):
            dma_in(t, xt_sb[:, :, t * TQ:(t + 1) * TQ],
                   xT.rearrange("(c p) t -> p c t", p=128)[:, :, t * TQ:(t + 1) * TQ])
        dma_in(6, wo_sb[:], wo.rearrange("(r p) n -> p r n", p=128))

        # ones columns of V' (only the 65th column of each head slot)
        vp65 = vp_sb.rearrange("p n (h m) -> p n h m", m=65)
        emit("vector", lambda e: nc.vector.memset(vp65[:, :, :, 64:65], 1.0),
             inc="dve")

        # PE warmup: dummy matmuls on (garbage) SBUF while input DMAs land,
        # so the clock-gate/p-state ramp finishes before phase A begins.
        # Results go to pO[0], whose first real use overwrites (start=True).
        for _ in range(36):
            emit("tensor", lambda e: nc.tensor.matmul(
                pO[0][:, 0:128], tmp_sb[:, 0, 0:128], tmp_sb[:, 0, 128:256],
                start=True, stop=True))

        # ---- phase A: qk projection + rope (per t-group) ----
        a_copy_done = {}   # i -> act cnt
        qk_read_done = {}  # i -> dve cnt after last qk-reading rope op
        rope_done = {}     # i -> dve cnt after rope add
        exp_par = {0: 0, 1: 0}  # par -> act cnt of last exp reading P2[par]

        def emit_A(t):
            wait("tensor", f"in{t}", 16)
            for m in range(4):
                i = t * 4 + m
                wait("tensor", "in4", 16 * (m + 1))
                if t > 0:
                    # bank last read by exp (C of previous group) on P2[m//2]
                    wait("tensor", "act", exp_par[m // 2])
                for c in range(KC):
                    emit("tensor",
                         lambda e, mm=m, cc=c, tt=t, ii=i: nc.tensor.matmul(
                             bankA(ii), wqk_sb[:, cc, mm * 128:(mm + 1) * 128],
                             xt_sb[:, cc, tt * TQ:(tt + 1) * TQ],
                             start=(cc == 0), stop=(cc == KC - 1)),
                         inc="pe" if c == KC - 1 else None)
                wait("scalar", "pe", cnt["pe"])
                if i >= 4:
                    wait("scalar", "dve", qk_read_done[i - 4])
                emit("scalar",
                     lambda e, ii=i: nc.scalar.copy(qk_sb[:, ii % 4, :], bankA(ii)),
                     inc="act")
                a_copy_done[i] = cnt["act"]
                # rope: 6 DVE ops
                wait("vector", "in7", 32)
                wait("vector", "act", a_copy_done[i])
                sl = slice(t * TQ, (t + 1) * TQ)
                par = i % 4
                emit("vector", lambda e, p2=par, s2=sl: nc.vector.tensor_mul(
                    tmp_sb[:, 0, :], qk_sb[:, p2, :], cos_sb[:, s2]))
                for hb in (0, 64):
                    emit("vector", lambda e, p2=par, s2=sl, h2=hb: nc.vector.tensor_mul(
                        tmp_sb[h2:h2 + 32, 1, :], qk_sb[h2 + 32:h2 + 64, p2, :],
                        sin_sb[h2 + 32:h2 + 64, s2]))
                    emit("vector", lambda e, p2=par, s2=sl, h2=hb: nc.vector.tensor_mul(
                        tmp_sb[h2 + 32:h2 + 64, 1, :], qk_sb[h2:h2 + 32, p2, :],
                        sin_sb[h2:h2 + 32, s2]),
                        inc="dve" if hb == 64 else None)
                qk_read_done[i] = cnt["dve"]
                emit("vector", lambda e, m2=m, s2=sl: nc.vector.tensor_add(
                    qkr_sb[:, m2, s2], tmp_sb[:, 0, :], tmp_sb[:, 1, :]),
                    inc="dve")
                rope_done[i] = cnt["dve"]

        # ---- phase B: V natural (+ones), per t-group ----
        b_copy_done = {}

        def emit_B(t):
            wait("tensor", "in5", 16)
            for tt in range(4 * t, 4 * t + 4):
                i = 16 + tt
                # bank tt%4 last used by A(t, m=tt%4) (causal interleave) or
                # by B(tt-4) (non-causal serial mode)
                wait("tensor", "act",
                     max(a_copy_done[4 * t + tt % 4], b_copy_done.get(tt - 4, 0)))
                for c in range(KC):
                    emit("tensor",
                         lambda e, cc=c, t2=tt, ii=i: nc.tensor.matmul(
                             bankA(ii)[:, 0:256],
                             xt_sb[:, cc, t2 * 128:(t2 + 1) * 128],
                             wv_sb[:, cc, :],
                             start=(cc == 0), stop=(cc == KC - 1)),
                         inc="pe" if c == KC - 1 else None)
                wait("scalar", "pe", cnt["pe"])
                if tt == 0:
                    wait("scalar", "dve", 1)  # ones memset
                emit("scalar",
                     lambda e, t2=tt, ii=i: nc.scalar.copy(
                         vp65[:, t2, :, 0:64],
                         bankA(ii)[:, 0:256].rearrange("p (h m) -> p h m", m=64)),
                     inc="act")
                b_copy_done[tt] = cnt["act"]

        # ---- phase C + interleaved phase D ----
        scale = 0.125
        pv_slot = {i: 0 for i in range(4)}  # p_sb slot -> pe cnt of last PV read
        po_read = {i: ("dve", 0) for i in range(4)}  # pO[i] -> (sem, cnt) of last read
        at2_ready = {}             # qt -> dve cnt after all 4 normalize mults
        rec_war = {0: 0, 1: 0}     # hh -> bc cnt after out-dma reading rec_sb slot
        rb_war = {}                # (buf,hh) -> dve cnt after mult reading rb slot
        out_war = {0: 0, 1: 0}     # tq%2 -> out sem cnt
        ob_war = {}                # (tqpar, n) -> dve cnt (copy) for DMA wait

        P4v = P4.rearrange("p (r h q) -> p r h q", h=2, q=512)
        P2v = [P4v[:, 0], P4v[:, 1]]

        def cols(qt, kj):
            r = kj - 4 * qt
            if causal and r >= 0:
                return 128 * r, 512 - 128 * r, True
            return 0, 512, False

        def emit_C(qt, hp):
            nkt_q = 4 * (qt + 1) if causal else NKT
            buf = hp
            qsl = slice(qt * TQ, (qt + 1) * TQ)
            exp_kj = {}
            mask_kj = {}

            def emit_S(kj):
                par = kj % 2
                coff, N, diag = cols(qt, kj)
                # rope deps: k tile (kj//4, m=2+hp); q tile (qt, m=hp)
                wait("tensor", "dve", rope_done[4 * (kj // 4) + 2 + hp])
                wait("tensor", "dve", rope_done[4 * qt + hp])
                wait("tensor", "act", exp_par[par])  # WAR on P2[par]
                for hh in (0, 1):
                    emit("tensor",
                         lambda e, h2=hh, k2=kj, c2=coff, n2=N: nc.tensor.matmul(
                             P2v[par][:, h2, c2:c2 + n2],
                             qkr_sb[64 * h2:64 * h2 + 64, 2 + hp,
                                    k2 * 128:(k2 + 1) * 128],
                             qkr_sb[64 * h2:64 * h2 + 64, hp,
                                    qt * TQ + c2:qt * TQ + c2 + n2],
                             start=True, stop=True),
                         inc="pe" if hh == 1 else None)
                return cnt["pe"]

            def emit_exp(kj, s_pe):
                # single-kj exp (diag tiles): suffix-trimmed, hh-fused
                par, slot = kj % 2, kj % 4
                coff, N, diag = cols(qt, kj)
                wait("scalar", "pe", s_pe)
                wait("scalar", "pe", pv_slot[slot])  # WAR on p_sb slot
                emit("scalar",
                     lambda e, c2=coff, n2=N: nc.scalar.activation(
                         p_sb[:, slot, :, c2:c2 + n2],
                         P2v[par][:, :, c2:c2 + n2], AF.Exp, scale=scale),
                     inc="act")
                exp_par[par] = cnt["act"]
                exp_kj[kj] = cnt["act"]
                if diag:
                    wait("vector", "in7", 48)
                    wait("vector", "act", cnt["act"])
                    for hh in (0, 1):
                        emit("vector",
                             lambda e, h2=hh, c2=coff: nc.vector.tensor_mul(
                                 p_sb[:, slot, h2, c2:c2 + 128],
                                 p_sb[:, slot, h2, c2:c2 + 128],
                                 dm_sb[:, :]),
                             inc="dve" if hh == 1 else None)
                    mask_kj[kj] = cnt["dve"]

            def emit_expF(kj, s_pe):
                # fused exp over (kj, kj+1) x both hh: all 4 P4 banks
                slot = kj % 4
                wait("scalar", "pe", s_pe)
                wait("scalar", "pe", max(pv_slot[slot], pv_slot[slot + 1]))
                emit("scalar",
                     lambda e, s2=slot: nc.scalar.activation(
                         p_sb[:, s2:s2 + 2, :, :], P4[:], AF.Exp, scale=scale),
                     inc="act")
                exp_par[0] = exp_par[1] = cnt["act"]
                exp_kj[kj] = exp_kj[kj + 1] = cnt["act"]

            def emit_PV(kj):
                slot = kj % 4
                coff, N, diag = cols(qt, kj)
                if diag:
                    wait("tensor", "dve", mask_kj[kj])
                else:
                    wait("tensor", "act", exp_kj[kj])
                if kj == 0:
                    for s2, v2 in (po_read[2 * buf], po_read[2 * buf + 1]):
                        wait("tensor", s2, v2)
                for hh in (0, 1):
                    h = 2 * hp + hh
                    emit("tensor",
                         lambda e, h2=hh, k2=kj, h3=h, c2=coff, n2=N,
                         last=(kj == nkt_q - 1): nc.tensor.matmul(
                             pO[2 * buf + h2][0:65, c2:c2 + n2],
                             vp_sb[:, k2, h3 * 65:(h3 + 1) * 65],
                             p_sb[:, k2 % 4, h2, c2:c2 + n2],
                             start=(k2 == 0), stop=last, skip_group_check=True),
                         inc="pe" if hh == 1 else None)
                pv_slot[slot] = cnt["pe"]

            # Software pipeline.  Non-diag kj come in pairs with one fused
            # exp over all 4 banks; PE order per pair:
            #   [expF gate] S(k+2) S(k+3) PV(k) PV(k+1)
            # Diag kj run singly: [exp gate] S(kj+2) PV(kj).
            s_pe = {}
            s_pe[0] = emit_S(0)
            if nkt_q > 1:
                s_pe[1] = emit_S(1)
            for kj in range(nkt_q):
                emit_exp(kj, s_pe[kj])
                if kj + 2 < nkt_q:
                    s_pe[kj + 2] = emit_S(kj + 2)
                emit_PV(kj)

            # normalization part 1 (inline): recips + one out-DMA + one
            # broadcast-DMA covering both hh.  The dependent multiplies are
            # deferred (emit_norm_mults) so the DVE stream never blocks on
            # the ~3us DMA round trip.
            wait("vector", "pe", pv_slot[(nkt_q - 1) % 4])
            wait("vector", "bc", rec_war[0])  # rec_sb slot WAR (out-dma read)
            for hh in (0, 1):
                emit("vector",
                     lambda e, i2=2 * buf + hh, h2=hh: nc.vector.reciprocal(
                         rec_sb[0:1, h2, :], pO[i2][64:65, :]),
                     inc="dve" if hh == 1 else None)
            wait("sync", "dve", cnt["dve"])
            emit("sync",
                 lambda e: e.dma_start(
                     out=rec_dram[buf, :, :], in_=rec_sb[0:1, :, :]),
                 inc="bc", inc_by=16)
            rec_war[0] = cnt["bc"]
            wait("sync", "bc", cnt["bc"])

            def _bcast_src(b2):
                a = rec_dram[b2, :, :]
                return bass.AP(tensor=a.tensor, offset=a.offset,
                               ap=[[0, 64], [TQ, 2], [1, TQ]])

            wait("sync", "dve", rb_war.get(buf, 0))  # rb slot WAR
            emit("sync",
                 lambda e, b2=buf: e.dma_start(
                     out=rb_sb[:, b2, :, :], in_=_bcast_src(b2)),
                 inc="bc", inc_by=16)
            return cnt["bc"]

        def emit_norm_mults(qt, hp, bc_after):
            buf = hp
            qsl = slice(qt * TQ, (qt + 1) * TQ)
            mults = []
            wait("vector", "bc", bc_after)
            for hh in (0, 1):
                i = 2 * buf + hh
                emit("vector",
                     lambda e, i2=i, h2=hh, b2=buf, q2=qsl: nc.vector.tensor_mul(
                         at2_sb[64 * h2:64 * h2 + 64, hp, q2],
                         pO[i2][0:64, :], rb_sb[:, b2, h2, :]),
                     inc="dve")
                po_read[i] = ("dve", cnt["dve"])
                mults.append(cnt["dve"])
            rb_war[buf] = cnt["dve"]
            return mults

        def emit_D(qt, four_banks=False):
            # out rows 512*qt .. 512*(qt+1); uses pO[2] (n=0, ACT copy) and
            # pO[3] (n=1, DVE copy) as banks; one out-DMA per 2 row-blocks.
            # four_banks (final batch): all pO banks free -> deeper rotation.
            wait("tensor", "in6", 16)
            for tq in range(4 * qt, 4 * qt + 4):
                sl4 = tq % 4
                for n in (0, 1):
                    bi = (2 * (tq % 2) + n) if four_banks else (2 + n)
                    bank = pO[bi]
                    s2, v2 = po_read[bi]
                    wait("tensor", s2, v2)
                    wait("tensor", "dve", at2_ready[qt])
                    for p in (0, 1):
                        emit("tensor",
                             lambda e, p2=p, t2=tq, n2=n, bk=bank: nc.tensor.matmul(
                                 bk[:],
                                 at2_sb[:, p2, t2 * 128:(t2 + 1) * 128],
                                 wo_sb[:, p2, n2 * 512:(n2 + 1) * 512],
                                 start=(p2 == 0), stop=(p2 == 1),
                                 skip_group_check=True),
                             inc="pe" if p == 1 else None)
                    ceng, csem = ("scalar", "act") if n == 0 else ("vector", "dve")
                    wait(ceng, "pe", cnt["pe"])
                    osem = f"out{(tq // 2) % 2}"
                    wait(ceng, osem, out_war[(tq // 2) % 2])  # ob slot WAR
                    if n == 0:
                        emit("scalar",
                             lambda e, s4=sl4, bk=bank: nc.scalar.copy(
                                 ob_sb[:, s4, 0, :], bk[:]),
                             inc="act")
                    else:
                        emit("vector",
                             lambda e, s4=sl4, bk=bank: nc.vector.tensor_copy(
                                 ob_sb[:, s4, 1, :], bk[:]),
                             inc="dve")
                    po_read[bi] = (csem, cnt[csem])
                    ob_war[(sl4, n)] = (csem, cnt[csem])
                if tq % 2 == 1:
                    # one DMA covering row-blocks tq-1, tq
                    op = (tq // 2) % 2
                    for s4 in (sl4 - 1, sl4):
                        for n in (0, 1):
                            s2, v2 = ob_war[(s4, n)]
                            wait("sync", s2, v2)
                    wait("sync", f"out{op}", out_war[op])
                    src = ob_sb[:, sl4 - 1:sl4 + 1, :, :]
                    dsl = out[(tq - 1) * 128:(tq + 1) * 128, :]
                    dst = bass.AP(tensor=dsl.tensor, offset=dsl.offset,
                                  ap=[[D, 128], [128 * D, 2], [1, D]])
                    emit("sync",
                         lambda e, s3=src, d3=dst: e.dma_start(out=d3, in_=s3),
                         inc=f"out{op}", inc_by=16)
                    out_war[op] = cnt[f"out{op}"]

        # Interleaved group sequence (causal): per t-group
        #   A(t), B(t), [mults(t-1,0)], C(t,0), [mults(t-1,1), D(t-1)], C(t,1)
        # C(t,*) only needs K/V tokens <= 512*(t+1), all produced by group t.
        # Non-causal C(qt) needs all K/V -> plain serial phases.
        bca = {}
        if causal:
            for t in range(NQT):
                emit_A(t)
                emit_B(t)
                exp_par[0] = max(exp_par[0], b_copy_done[4 * t + 1])
                exp_par[1] = max(exp_par[1], b_copy_done[4 * t + 3])
                if t > 0:
                    # both mult sets here: their bcast DMAs completed during
                    # A(t)/B(t), and the DVE ropes have drained by now
                    m0 = emit_norm_mults(t - 1, 0, bca[(t - 1, 0)])
                    m1 = emit_norm_mults(t - 1, 1, bca[(t - 1, 1)])
                    at2_ready[t - 1] = max(m0 + m1)
                bca[(t, 0)] = emit_C(t, 0)
                if t > 0:
                    emit_D(t - 1)
                bca[(t, 1)] = emit_C(t, 1)
        else:
            for t in range(NQT):
                emit_A(t)
            for t in range(NQT):
                emit_B(t)
            exp_par[0] = max(exp_par[0], b_copy_done[13])
            exp_par[1] = max(exp_par[1], b_copy_done[15])
            for qt in range(NQT):
                if qt > 0:
                    m0 = emit_norm_mults(qt - 1, 0, bca[(qt - 1, 0)])
                bca[(qt, 0)] = emit_C(qt, 0)
                if qt > 0:
                    m1 = emit_norm_mults(qt - 1, 1, bca[(qt - 1, 1)])
                    at2_ready[qt - 1] = max(m0 + m1)
                    emit_D(qt - 1)
                bca[(qt, 1)] = emit_C(qt, 1)
        m0 = emit_norm_mults(NQT - 1, 0, bca[(NQT - 1, 0)])
        m1 = emit_norm_mults(NQT - 1, 1, bca[(NQT - 1, 1)])
        at2_ready[NQT - 1] = max(m0 + m1)
        emit_D(NQT - 1)

        # ---- final drains ----
        wait("sync", "out0", cnt["out0"])
        wait("sync", "out1", cnt["out1"])
        wait("sync", "bc", cnt["bc"])

        # ---------- emit per-engine programs ----------
        def runner(name):
            def _run(eng):
                for e_name, fn in sched:
                    if e_name == name:
                        fn(eng)
            return _run

        block.tensor(runner("tensor"))
        block.scalar(runner("scalar"))
        block.vector(runner("vector"))
        block.sync(runner("sync"))

    return nc


_NC_CACHE = {}
_RUN_KWARGS = {}
_LAST_RESULT = None


def _get_nc(causal: bool):
    if causal not in _NC_CACHE:
        _NC_CACHE[causal] = _build_nc(causal)
    return _NC_CACHE[causal]


def _host_inputs(x, Wqkv, Wout, cos, sin):
    import ml_dtypes
    bf16 = ml_dtypes.bfloat16
    c = np.ascontiguousarray(cos.T)          # [32, T]
    s = np.ascontiguousarray(sin.T)
    cos2 = np.tile(c, (4, 1)).astype(bf16)   # [128, T]
    sinF = np.concatenate([s, -s, s, -s], axis=0).astype(bf16)
    dm1 = (np.arange(128)[:, None] <= np.arange(128)[None, :]).astype(bf16)
    Wq, Wk, Wv = Wqkv[:, 0:D], Wqkv[:, D:2 * D], Wqkv[:, 2 * D:3 * D]
    in_maps = []
    for core in range(8):
        b, g = divmod(core, NG)
        hs = slice(g * HPC * DH, (g + 1) * HPC * DH)
        in_maps.append({
            "xT": np.ascontiguousarray(x[b].T).astype(bf16),
            "wqk": np.concatenate([Wq[:, hs], Wk[:, hs]], axis=1).astype(bf16),
            "wv": np.ascontiguousarray(Wv[:, hs]).astype(bf16),
            "wo": np.ascontiguousarray(Wout[hs, :]).astype(bf16),
            "cos2": cos2,
            "sinF": sinF,
            "dm1": dm1,
        })
    return in_maps


def kernel(x, Wqkv, Wout, cos, sin, mask):
    import sys
    if "/opt/trn_rl_repo" not in sys.path:
        sys.path.insert(0, "/opt/trn_rl_repo")
    from concourse.bass_utils import run_bass_kernel_spmd

    x = np.asarray(x)
    mask = np.asarray(mask)
    m2 = mask.reshape(T, T)
    causal = bool(np.array_equal(m2, np.tril(np.ones((T, T), dtype=bool))))
    if not causal:
        assert m2.all(), "only causal or all-ones masks supported"

    in_maps = _host_inputs(x, np.asarray(Wqkv), np.asarray(Wout),
                           np.asarray(cos), np.asarray(sin))
    nc = _get_nc(causal)
    res = run_bass_kernel_spmd(nc, in_maps, list(range(8)), **_RUN_KWARGS)
    global _LAST_RESULT
    _LAST_RESULT = res
    outs = [np.asarray(r["out"], dtype=np.float32) for r in res.results]
    return np.stack([outs[0] + outs[1] + outs[2] + outs[3],
                     outs[4] + outs[5] + outs[6] + outs[7]])


# revision 7
# speedup vs baseline: 1.0683x; 1.0683x over previous
"""Distributed causal MHA + RoPE kernel for 8 TRN2 NeuronCores (raw Bass), v2.

Reference (B=2, T=2048, D=1024, H=16, DH=64):
    qkv = x @ Wqkv -> per-head q,k,v -> RoPE(q,k)
    attn = softmax(causal(q k^T / 8)) @ v ;  out = concat_heads(attn) @ Wout

Sharding: 8 cores = 2 batches x 4 head-groups (4 heads each). Each core
emits a bf16 partial of the out-projection (its heads' rows of Wout);
the 4 partials per batch are summed on the host.

v2 structure (vs v1):
 - phases interleaved per 512-token group t: A(t) qk-proj+rope, B(t) V,
   C(t,hp=0/1) attention for query block t, D(t-1) out-projection --
   causal attention for query block t only needs K/V tokens <= 512(t+1).
 - phase C software pipeline: PE stream runs [exp-gate, S(kj+2) pair,
   PV(kj) pair] per kj; S^T tiles double-buffered across 4 PSUM banks
   (P4, kj parity x head-in-pair), exp fused over both heads in one ACT
   instruction reading 2 adjacent banks; causal suffix-trim shrinks
   S/exp/PV to the unmasked column range; triangular mask multiply only
   on the [128,128] diagonal block (DVE).
 - rope in 6 wide DVE ops per tile via sign-folded sin tables and
   partition-shifted writes (out = qk*cos + swap32(qk*sinF)).
 - softmax denominators from a ones-column in V' (PV M=65); the
   normalize 1/d column-broadcast goes through a DRAM round trip whose
   dependent multiplies are emitted one sub-phase later, keeping the
   DVE stream from blocking; PV accumulators double-buffered by hp.
 - phase D: K=128 head-pair matmuls, PSUM->bf16 copies split ACT/DVE,
   two row-blocks per output DMA; bf16 partials summed on host.
 - PE warmup matmuls during the input DMAs (clock-gate ramp).
"""

import numpy as np

B, T, D, H, DH = 2, 2048, 1024, 16, 64
HPC = 4            # heads per core
NG = 4             # head groups
TQ = 512
NQT = T // TQ      # 4
NKT = T // 128     # 16
KC = D // 128      # 8


def _build_nc(causal: bool):
    import concourse.bass as bass
    import concourse.mybir as mybir
    from contextlib import ExitStack

    dt = mybir.dt
    f32, bf16 = dt.float32, dt.bfloat16
    AF = mybir.ActivationFunctionType
    nc = bass.Bass()

    xT = nc.declare_dram_parameter("xT", [D, T], bf16, isOutput=False)
    wqk = nc.declare_dram_parameter("wqk", [D, 512], bf16, isOutput=False)
    wv = nc.declare_dram_parameter("wv", [D, 256], bf16, isOutput=False)
    wo = nc.declare_dram_parameter("wo", [256, D], bf16, isOutput=False)
    cs = nc.declare_dram_parameter("cs", [128, 2 * T], bf16, isOutput=False)
    dm1 = nc.declare_dram_parameter("dm1", [128, 128], bf16, isOutput=False)
    out = nc.declare_dram_parameter("out", [T, D], bf16, isOutput=True)
    rec_dram = nc.dram_tensor("rec_dram", [2, 2, TQ], f32)

    ctx = ExitStack()
    with ctx:
        sb = lambda name, shape, dtype: ctx.enter_context(
            nc.sbuf_tensor(name, shape, dtype))
        ps = lambda name, shape: ctx.enter_context(
            nc.psum_tensor(name, shape, f32))

        wqk_sb = sb("wqk_sb", [128, KC, 512], bf16)
        wv_sb = sb("wv_sb", [128, KC, 256], bf16)
        wo_sb = sb("wo_sb", [128, 2, D], bf16)       # head pairs stacked on K
        cs_sb = sb("cs_sb", [128, 2, T], bf16)       # [cos | sign-folded sin]
        dm_sb = sb("dm_sb", [128, 128], bf16)        # triangular k<=q
        xt_sb = sb("xt_sb", [128, KC, T], bf16)
        qk_sb = sb("qk_sb", [128, 4, TQ], bf16)      # A-copy staging (4-deep)
        tmp_sb = sb("tmp_sb", [128, 2, TQ], bf16)    # rope cos-prod / sin-prod
        qkr_sb = sb("qkr_sb", [128, 4, T], bf16)     # post-rope q01,q23,k01,k23
        vp_sb = sb("vp_sb", [128, NKT, HPC * 65], bf16)
        p_sb = sb("p_sb", [128, 4, 2, TQ], bf16)     # (kj%4 slot, hh)
        at2_sb = sb("at2_sb", [128, 2, T], bf16)     # o^T, head pairs on K
        rec_sb = sb("rec_sb", [1, 2, TQ], f32)
        ones_sb = sb("ones_sb", [1, 64], f32)        # K=1 broadcast stationary
        rb_sb = sb("rb_sb", [64, 2, 2, TQ], f32)     # (buf, hh)
        ob_sb = sb("ob_sb", [128, 4, 2, 512], bf16)  # (tq%4, n)

        # PSUM: 8 banks total.  P4 = 4 adjacent banks: (parity, hh) halves
        # for S^T tiles -- adjacency lets one ACT exp cover 2 kj x 2 hh.
        # pO[2*buf+hh] 1 bank each for PV accum; phases A/B/D reuse these
        # banks (A/B: 4 banks of P4; D: pO[2], pO[3]).
        P4 = ps("P4", [128, 2048])
        pO = [ps(f"pO{i}", [128, 512]) for i in range(4)]

        sem_names = (["pe", "act", "dve", "bc", "out0", "out1"]
                     + [f"in{i}" for i in range(8)])
        sems = {n: ctx.enter_context(nc.semaphore(f"s_{n}")) for n in sem_names}
        block = ctx.enter_context(nc.Block())

        # ---------- schedule construction ----------
        sched = []  # (engine, fn)
        cnt = {n: 0 for n in sem_names}
        last_wait = {}
        pend = {}   # eng -> [(sem_handle, val)] waits not yet attached

        def wait(eng, sem, val):
            if val <= 0:
                return
            key = (eng, sem)
            if last_wait.get(key, -1) >= val:
                return
            last_wait[key] = val
            pend.setdefault(eng, []).append((sems[sem], val))

        def emit(eng, fn, inc=None, inc_by=1):
            # attach the last pending wait to the instruction itself (one
            # sync_info wait slot); extras become standalone EventSemaphores
            pw = pend.pop(eng, None)
            att = None
            if pw:
                att = pw[-1]
                for so, v in pw[:-1]:
                    sched.append((eng, lambda e, s=so, v2=v: e.wait_ge(s, v2)))
            si = None
            if inc is not None:
                si = sems[inc]
                cnt[inc] += inc_by

            def run(e, f=fn, a=att, s=si, ib=inc_by):
                inst = f(e)
                if a is not None:
                    inst = inst._wait_ge(a[0], a[1])
                if s is not None:
                    inst.then_inc(s, ib)
            sched.append((eng, run))

        def flush_waits():
            for eng, lst in list(pend.items()):
                for so, v in lst:
                    sched.append((eng, lambda e, s=so, v2=v: e.wait_ge(s, v2)))
            pend.clear()

        def bankA(i):
            return P4.rearrange("p (b q) -> p b q", q=512)[:, i % 4, :]

        # ---- input DMAs (sync engine queue, ordered by first use) ----
        def dma_in(sem_i, dst, src):
            emit("sync", lambda e, d=dst, s=src: e.dma_start(out=d, in_=s),
                 inc=f"in{sem_i}", inc_by=16)

        wqk_r = wqk.rearrange("(c p) m -> p c m", p=128)
        dma_in(4, wqk_sb[:, :, 0:128], wqk_r[:, :, 0:128])
        dma_in(0, xt_sb[:, :, 0:TQ], xT.rearrange("(c p) t -> p c t", p=128)[:, :, 0:TQ])
        dma_in(4, wqk_sb[:, :, 128:512], wqk_r[:, :, 128:512])
        dma_in(7, cs_sb[:], cs[:])
        wv_r = wv.rearrange("(c p) m -> p c m", p=128)
        dma_in(5, wv_sb[:, 0:4, :], wv_r[:, 0:4, :])
        dma_in(5, wv_sb[:, 4:8, :], wv_r[:, 4:8, :])
        dma_in(1, xt_sb[:, :, TQ:2 * TQ],
               xT.rearrange("(c p) t -> p c t", p=128)[:, :, TQ:2 * TQ])
        dma_in(7, dm_sb[:], dm1[:])
        for t in range(2, 4):
            dma_in(t, xt_sb[:, :, t * TQ:(t + 1) * TQ],
                   xT.rearrange("(c p) t -> p c t", p=128)[:, :, t * TQ:(t + 1) * TQ])
        dma_in(6, wo_sb[:], wo.rearrange("(r p) n -> p r n", p=128))

        # ones columns of V' (only the 65th column of each head slot)
        vp65 = vp_sb.rearrange("p n (h m) -> p n h m", m=65)
        emit("vector", lambda e: nc.vector.memset(vp65[:, :, :, 64:65], 1.0),
             inc="dve")
        emit("vector", lambda e: nc.vector.memset(ones_sb[:], 1.0))

        # PE warmup: dummy matmuls on (garbage) SBUF while input DMAs land,
        # so the clock-gate/p-state ramp finishes before phase A begins.
        # Results go to pO[0], whose first real use overwrites (start=True).
        for _ in range(56):
            emit("tensor", lambda e: nc.tensor.matmul(
                pO[0][:, 0:128], tmp_sb[:, 0, 0:128], tmp_sb[:, 0, 128:256],
                start=True, stop=True))

        # ---- phase A: qk projection + rope (per t-group) ----
        a_copy_done = {}   # i -> act cnt
        qk_read_done = {}  # i -> dve cnt after last qk-reading rope op
        rope_done = {}     # i -> dve cnt after rope add
        exp_par = {0: 0, 1: 0}  # par -> act cnt of last exp reading P2[par]

        def emit_A(t):
            wait("tensor", f"in{t}", 16)
            for m in range(4):
                i = t * 4 + m
                wait("tensor", "in4", 16 if m == 0 else 32)
                if t > 0:
                    # bank last read by exp (C of previous group) on P2[m//2]
                    wait("tensor", "act", exp_par[m // 2])
                for c in range(KC):
                    emit("tensor",
                         lambda e, mm=m, cc=c, tt=t, ii=i: nc.tensor.matmul(
                             bankA(ii), wqk_sb[:, cc, mm * 128:(mm + 1) * 128],
                             xt_sb[:, cc, tt * TQ:(tt + 1) * TQ],
                             start=(cc == 0), stop=(cc == KC - 1)),
                         inc="pe" if c == KC - 1 else None)
                wait("scalar", "pe", cnt["pe"])
                if i >= 4:
                    wait("scalar", "dve", qk_read_done[i - 4])
                emit("scalar",
                     lambda e, ii=i: nc.scalar.copy(qk_sb[:, ii % 4, :], bankA(ii)),
                     inc="act")
                a_copy_done[i] = cnt["act"]
                # rope: 6 DVE ops
                wait("vector", "in7", 16)
                wait("vector", "act", a_copy_done[i])
                sl = slice(t * TQ, (t + 1) * TQ)
                par = i % 4
                emit("vector", lambda e, p2=par, s2=sl: nc.vector.tensor_mul(
                    tmp_sb[:, 0, :], qk_sb[:, p2, :], cs_sb[:, 0, s2]))
                for hb in (0, 64):
                    emit("vector", lambda e, p2=par, s2=sl, h2=hb: nc.vector.tensor_mul(
                        tmp_sb[h2:h2 + 32, 1, :], qk_sb[h2 + 32:h2 + 64, p2, :],
                        cs_sb[h2 + 32:h2 + 64, 1, s2]))
                    emit("vector", lambda e, p2=par, s2=sl, h2=hb: nc.vector.tensor_mul(
                        tmp_sb[h2 + 32:h2 + 64, 1, :], qk_sb[h2:h2 + 32, p2, :],
                        cs_sb[h2:h2 + 32, 1, s2]),
                        inc="dve" if hb == 64 else None)
                qk_read_done[i] = cnt["dve"]
                emit("vector", lambda e, m2=m, s2=sl: nc.vector.tensor_add(
                    qkr_sb[:, m2, s2], tmp_sb[:, 0, :], tmp_sb[:, 1, :]),
                    inc="dve")
                rope_done[i] = cnt["dve"]

        # ---- phase B: V natural (+ones), per t-group ----
        b_copy_done = {}

        def emit_B(t):
            wait("tensor", "in5", 32)
            for tt in range(4 * t, 4 * t + 4):
                i = 16 + tt
                # bank tt%4 last used by A(t, m=tt%4) (causal interleave) or
                # by B(tt-4) (non-causal serial mode)
                wait("tensor", "act",
                     max(a_copy_done[4 * t + tt % 4], b_copy_done.get(tt - 4, 0)))
                for c in range(KC):
                    emit("tensor",
                         lambda e, cc=c, t2=tt, ii=i: nc.tensor.matmul(
                             bankA(ii)[:, 0:256],
                             xt_sb[:, cc, t2 * 128:(t2 + 1) * 128],
                             wv_sb[:, cc, :],
                             start=(cc == 0), stop=(cc == KC - 1)),
                         inc="pe" if c == KC - 1 else None)
                wait("scalar", "pe", cnt["pe"])
                if tt == 0:
                    wait("scalar", "dve", 1)  # ones memset
                emit("scalar",
                     lambda e, t2=tt, ii=i: nc.scalar.copy(
                         vp65[:, t2, :, 0:64],
                         bankA(ii)[:, 0:256].rearrange("p (h m) -> p h m", m=64)),
                     inc="act")
                b_copy_done[tt] = cnt["act"]

        # ---- phase C + interleaved phase D ----
        scale = 0.125
        pv_slot = {i: 0 for i in range(4)}  # p_sb slot -> pe cnt of last PV read
        po_read = {i: ("dve", 0) for i in range(4)}  # pO[i] -> (sem, cnt) of last read
        at2_ready = {}             # qt -> dve cnt after all 4 normalize mults
        rec_war = {0: 0, 1: 0}     # hh -> bc cnt after out-dma reading rec_sb slot
        rb_war = {}                # (buf,hh) -> dve cnt after mult reading rb slot
        out_war = {0: 0, 1: 0}     # tq%2 -> out sem cnt
        ob_war = {}                # (tqpar, n) -> dve cnt (copy) for DMA wait

        P4v = P4.rearrange("p (r h q) -> p r h q", h=2, q=512)
        P2v = [P4v[:, 0], P4v[:, 1]]

        def cols(qt, kj):
            r = kj - 4 * qt
            if causal and r >= 0:
                return 128 * r, 512 - 128 * r, True
            return 0, 512, False

        def emit_C(qt, hp, pe_bcast=False):
            nkt_q = 4 * (qt + 1) if causal else NKT
            buf = hp
            qsl = slice(qt * TQ, (qt + 1) * TQ)
            exp_kj = {}
            mask_kj = {}

            def emit_S(kj):
                par = kj % 2
                coff, N, diag = cols(qt, kj)
                # rope deps: k tile (kj//4, m=2+hp); q tile (qt, m=hp)
                wait("tensor", "dve", rope_done[4 * (kj // 4) + 2 + hp])
                wait("tensor", "dve", rope_done[4 * qt + hp])
                wait("tensor", "act", exp_par[par])  # WAR on P2[par]
                for hh in (0, 1):
                    emit("tensor",
                         lambda e, h2=hh, k2=kj, c2=coff, n2=N: nc.tensor.matmul(
                             P2v[par][:, h2, c2:c2 + n2],
                             qkr_sb[64 * h2:64 * h2 + 64, 2 + hp,
                                    k2 * 128:(k2 + 1) * 128],
                             qkr_sb[64 * h2:64 * h2 + 64, hp,
                                    qt * TQ + c2:qt * TQ + c2 + n2],
                             start=True, stop=True),
                         inc="pe" if hh == 1 else None)
                return cnt["pe"]

            def emit_exp(kj, s_pe):
                # single-kj exp (diag tiles): suffix-trimmed, hh-fused
                par, slot = kj % 2, kj % 4
                coff, N, diag = cols(qt, kj)
                wait("scalar", "pe", s_pe)
                wait("scalar", "pe", pv_slot[slot])  # WAR on p_sb slot
                emit("scalar",
                     lambda e, c2=coff, n2=N: nc.scalar.activation(
                         p_sb[:, slot, :, c2:c2 + n2],
                         P2v[par][:, :, c2:c2 + n2], AF.Exp, scale=scale),
                     inc="act")
                exp_par[par] = cnt["act"]
                exp_kj[kj] = cnt["act"]
                if diag:
                    wait("vector", "in7", 32)
                    wait("vector", "act", cnt["act"])
                    for hh in (0, 1):
                        emit("vector",
                             lambda e, h2=hh, c2=coff: nc.vector.tensor_mul(
                                 p_sb[:, slot, h2, c2:c2 + 128],
                                 p_sb[:, slot, h2, c2:c2 + 128],
                                 dm_sb[:, :]),
                             inc="dve" if hh == 1 else None)
                    mask_kj[kj] = cnt["dve"]

            def emit_expF(kj, s_pe):
                # fused exp over (kj, kj+1) x both hh: all 4 P4 banks
                slot = kj % 4
                wait("scalar", "pe", s_pe)
                wait("scalar", "pe", max(pv_slot[slot], pv_slot[slot + 1]))
                emit("scalar",
                     lambda e, s2=slot: nc.scalar.activation(
                         p_sb[:, s2:s2 + 2, :, :], P4[:], AF.Exp, scale=scale),
                     inc="act")
                exp_par[0] = exp_par[1] = cnt["act"]
                exp_kj[kj] = exp_kj[kj + 1] = cnt["act"]

            def emit_PV(kj):
                slot = kj % 4
                coff, N, diag = cols(qt, kj)
                if diag:
                    wait("tensor", "dve", mask_kj[kj])
                else:
                    wait("tensor", "act", exp_kj[kj])
                if kj == 0:
                    for s2, v2 in (po_read[2 * buf], po_read[2 * buf + 1]):
                        wait("tensor", s2, v2)
                for hh in (0, 1):
                    h = 2 * hp + hh
                    emit("tensor",
                         lambda e, h2=hh, k2=kj, h3=h, c2=coff, n2=N,
                         last=(kj == nkt_q - 1): nc.tensor.matmul(
                             pO[2 * buf + h2][0:65, c2:c2 + n2],
                             vp_sb[:, k2, h3 * 65:(h3 + 1) * 65],
                             p_sb[:, k2 % 4, h2, c2:c2 + n2],
                             start=(k2 == 0), stop=last, skip_group_check=True),
                         inc="pe" if hh == 1 else None)
                pv_slot[slot] = cnt["pe"]

            # Software pipeline.  Non-diag kj come in pairs with one fused
            # exp over all 4 banks; PE order per pair:
            #   [expF gate] S(k+2) S(k+3) PV(k) PV(k+1)
            # Diag kj run singly: [exp gate] S(kj+2) PV(kj).
            s_pe = {}
            s_pe[0] = emit_S(0)
            if nkt_q > 1:
                s_pe[1] = emit_S(1)
            for kj in range(nkt_q):
                emit_exp(kj, s_pe[kj])
                if kj + 2 < nkt_q:
                    s_pe[kj + 2] = emit_S(kj + 2)
                emit_PV(kj)

            # normalization part 1 (inline): recips + one out-DMA + one
            # broadcast-DMA covering both hh.  The dependent multiplies are
            # deferred (emit_norm_mults) so the DVE stream never blocks on
            # the ~3us DMA round trip.  pe_bcast (final sub-phase): bf16
            # recips only; the broadcast happens via a K=1 matmul in the
            # tail instead of the DRAM round trip.
            wait("vector", "pe", pv_slot[(nkt_q - 1) % 4])
            if pe_bcast:
                for hh in (0, 1):
                    emit("vector",
                         lambda e, i2=2 * buf + hh, h2=hh: nc.vector.reciprocal(
                             rec_sb[0:1, h2, :], pO[i2][64:65, :]),
                         inc="dve" if hh == 1 else None)
                return cnt["dve"]
            wait("vector", "bc", rec_war[0])  # rec_sb slot WAR (out-dma read)
            for hh in (0, 1):
                emit("vector",
                     lambda e, i2=2 * buf + hh, h2=hh: nc.vector.reciprocal(
                         rec_sb[0:1, h2, :], pO[i2][64:65, :]),
                     inc="dve" if hh == 1 else None)
            wait("sync", "dve", cnt["dve"])
            emit("sync",
                 lambda e: e.dma_start(
                     out=rec_dram[buf, :, :], in_=rec_sb[0:1, :, :]),
                 inc="bc", inc_by=16)
            rec_war[0] = cnt["bc"]
            wait("sync", "bc", cnt["bc"])

            def _bcast_src(b2):
                a = rec_dram[b2, :, :]
                return bass.AP(tensor=a.tensor, offset=a.offset,
                               ap=[[0, 64], [TQ, 2], [1, TQ]])

            wait("sync", "dve", rb_war.get(buf, 0))  # rb slot WAR
            emit("sync",
                 lambda e, b2=buf: e.dma_start(
                     out=rb_sb[:, b2, :, :], in_=_bcast_src(b2)),
                 inc="bc", inc_by=16)
            return cnt["bc"]

        def emit_norm_mults(qt, hp, bc_after):
            buf = hp
            qsl = slice(qt * TQ, (qt + 1) * TQ)
            mults = []
            wait("vector", "bc", bc_after)
            for hh in (0, 1):
                i = 2 * buf + hh
                emit("vector",
                     lambda e, i2=i, h2=hh, b2=buf, q2=qsl: nc.vector.tensor_mul(
                         at2_sb[64 * h2:64 * h2 + 64, hp, q2],
                         pO[i2][0:64, :], rb_sb[:, b2, h2, :]),
                     inc="dve")
                po_read[i] = ("dve", cnt["dve"])
                mults.append(cnt["dve"])
            rb_war[buf] = cnt["dve"]
            return mults

        def emit_D(qt, four_banks=False):
            # out rows 512*qt .. 512*(qt+1); uses pO[2] (n=0, ACT copy) and
            # pO[3] (n=1, DVE copy) as banks; one out-DMA per 2 row-blocks.
            # four_banks (final batch): all pO banks free -> deeper rotation.
            wait("tensor", "in6", 16)
            for tq in range(4 * qt, 4 * qt + 4):
                sl4 = tq % 4
                for n in (0, 1):
                    bi = (2 * (tq % 2) + n) if four_banks else (2 + n)
                    bank = pO[bi]
                    s2, v2 = po_read[bi]
                    wait("tensor", s2, v2)
                    wait("tensor", "dve", at2_ready[qt])
                    for p in (0, 1):
                        emit("tensor",
                             lambda e, p2=p, t2=tq, n2=n, bk=bank: nc.tensor.matmul(
                                 bk[:],
                                 at2_sb[:, p2, t2 * 128:(t2 + 1) * 128],
                                 wo_sb[:, p2, n2 * 512:(n2 + 1) * 512],
                                 start=(p2 == 0), stop=(p2 == 1),
                                 skip_group_check=True),
                             inc="pe" if p == 1 else None)
                    ceng, csem = ("scalar", "act") if n == 0 else ("vector", "dve")
                    wait(ceng, "pe", cnt["pe"])
                    osem = f"out{(tq // 2) % 2}"
                    wait(ceng, osem, out_war[(tq // 2) % 2])  # ob slot WAR
                    if n == 0:
                        emit("scalar",
                             lambda e, s4=sl4, bk=bank: nc.scalar.copy(
                                 ob_sb[:, s4, 0, :], bk[:]),
                             inc="act")
                    else:
                        emit("vector",
                             lambda e, s4=sl4, bk=bank: nc.vector.tensor_copy(
                                 ob_sb[:, s4, 1, :], bk[:]),
                             inc="dve")
                    po_read[bi] = (csem, cnt[csem])
                    ob_war[(sl4, n)] = (csem, cnt[csem])
                if tq % 2 == 1:
                    # one DMA covering row-blocks tq-1, tq
                    op = (tq // 2) % 2
                    for s4 in (sl4 - 1, sl4):
                        for n in (0, 1):
                            s2, v2 = ob_war[(s4, n)]
                            wait("sync", s2, v2)
                    wait("sync", f"out{op}", out_war[op])
                    src = ob_sb[:, sl4 - 1:sl4 + 1, :, :]
                    dsl = out[(tq - 1) * 128:(tq + 1) * 128, :]
                    dst = bass.AP(tensor=dsl.tensor, offset=dsl.offset,
                                  ap=[[D, 128], [128 * D, 2], [1, D]])
                    emit("sync",
                         lambda e, s3=src, d3=dst: e.dma_start(out=d3, in_=s3),
                         inc=f"out{op}", inc_by=16)
                    out_war[op] = cnt[f"out{op}"]

        # Interleaved group sequence (causal): per t-group
        #   A(t), B(t), [mults(t-1,0)], C(t,0), [mults(t-1,1), D(t-1)], C(t,1)
        # C(t,*) only needs K/V tokens <= 512*(t+1), all produced by group t.
        # Non-causal C(qt) needs all K/V -> plain serial phases.
        bca = {}
        if causal:
            for t in range(NQT):
                emit_A(t)
                emit_B(t)
                exp_par[0] = max(exp_par[0], b_copy_done[4 * t + 1])
                exp_par[1] = max(exp_par[1], b_copy_done[4 * t + 3])
                if t > 0:
                    # both mult sets here: their bcast DMAs completed during
                    # A(t)/B(t), and the DVE ropes have drained by now
                    m0 = emit_norm_mults(t - 1, 0, bca[(t - 1, 0)])
                    m1 = emit_norm_mults(t - 1, 1, bca[(t - 1, 1)])
                    at2_ready[t - 1] = max(m0 + m1)
                bca[(t, 0)] = emit_C(t, 0)
                if t > 0:
                    emit_D(t - 1)
                if t == NQT - 1:
                    # final-group early normalize of (t,0): its bcast lands
                    # mid-C(t,1); the masks it could block only start at
                    # kj=4t, far later.  Frees pO[0]/pO[1] for the split
                    # final out-projection right after the last PV.
                    m0_final = emit_norm_mults(t, 0, bca[(t, 0)])
                bca[(t, 1)] = emit_C(t, 1, pe_bcast=(t == NQT - 1 and causal))
        else:
            for t in range(NQT):
                emit_A(t)
            for t in range(NQT):
                emit_B(t)
            exp_par[0] = max(exp_par[0], b_copy_done[13])
            exp_par[1] = max(exp_par[1], b_copy_done[15])
            for qt in range(NQT):
                if qt > 0:
                    m0 = emit_norm_mults(qt - 1, 0, bca[(qt - 1, 0)])
                bca[(qt, 0)] = emit_C(qt, 0)
                if qt > 0:
                    m1 = emit_norm_mults(qt - 1, 1, bca[(qt - 1, 1)])
                    at2_ready[qt - 1] = max(m0 + m1)
                    emit_D(qt - 1)
                bca[(qt, 1)] = emit_C(qt, 1)
            m0_final = emit_norm_mults(NQT - 1, 0, bca[(NQT - 1, 0)])
        # ---- final out-projection, split by head pair ----
        # After the last sub-phase the P4 banks are free (exps done) and
        # pO[0]/pO[1] free after mults(3,0).  Pair-0 matmuls for tq 12,13
        # run immediately; the (3,1) normalize uses a K=1 ones-matmul
        # broadcast of the bf16 reciprocals (no DRAM round trip), then
        # pair-1 accumulates and tq14/15 run in full.
        wait("tensor", "in6", 16)
        m0 = m0_final
        fbank = {}
        for tq in (12, 13):
            for n in (0, 1):
                fbank[(tq, n)] = bankA((tq - 12) * 2 + n)
        fbank[(14, 0)], fbank[(14, 1)] = pO[0][:], pO[1][:]
        fbank[(15, 0)], fbank[(15, 1)] = pO[2][:], pO[3][:]
        d_stop = {}

        def d_mm(tq, n, p, start, stop):
            emit("tensor",
                 lambda e, p2=p, t2=tq, bk=fbank[(tq, n)], n2=n: nc.tensor.matmul(
                     bk, at2_sb[:, p2, t2 * 128:(t2 + 1) * 128],
                     wo_sb[:, p2, n2 * 512:(n2 + 1) * 512],
                     start=start, stop=stop, skip_group_check=True),
                 inc="pe")
            if stop:
                d_stop[(tq, n)] = cnt["pe"]

        wait("tensor", "act", max(exp_par[0], exp_par[1]))  # P4 banks free
        wait("tensor", "dve", max(m0))                      # at2 pair0 ready
        for tq in (12, 13):
            for n in (0, 1):
                d_mm(tq, n, 0, True, False)
        qsl3 = slice((NQT - 1) * TQ, NQT * TQ)
        if causal:
            # K=1 broadcast of recb into pO[0]/pO[1] rows 0..64
            wait("tensor", "dve", bca[(NQT - 1, 1)])  # recips done
            for hh in (0, 1):
                emit("tensor",
                     lambda e, h2=hh: nc.tensor.matmul(
                         pO[h2][0:64, :], ones_sb[0:1, :], rec_sb[0:1, h2, :],
                         start=True, stop=True, skip_group_check=True),
                     inc="pe")
            wait("scalar", "pe", cnt["pe"])
            wait("scalar", "dve", max(m0))  # rb slot read by mults(3,0)
            for hh in (0, 1):
                emit("scalar",
                     lambda e, h2=hh: nc.scalar.copy(
                         rb_sb[:, 0, h2, :], pO[h2][0:64, :]),
                     inc="act")
            rbc = cnt["act"]
            wait("vector", "act", rbc)
            m1 = []
            for hh in (0, 1):
                emit("vector",
                     lambda e, h2=hh: nc.vector.tensor_mul(
                         at2_sb[64 * h2:64 * h2 + 64, 1, qsl3],
                         pO[2 + h2][0:64, :], rb_sb[0:64, 0, h2, :]),
                     inc="dve")
                m1.append(cnt["dve"])
            wait("tensor", "act", rbc)  # pO[0]/pO[1] freed by the copies
        else:
            m1 = emit_norm_mults(NQT - 1, 1, bca[(NQT - 1, 1)])
        wait("tensor", "dve", max(m1))  # at2 pair1 ready (and pO2/3 free)
        for tq in (12, 13):
            for n in (0, 1):
                d_mm(tq, n, 1, False, True)
        for n in (0, 1):
            d_mm(14, n, 0, True, False)
            d_mm(14, n, 1, False, True)
        for n in (0, 1):
            d_mm(15, n, 0, True, False)
            d_mm(15, n, 1, False, True)
        for tq in (12, 13, 14, 15):
            sl4 = tq % 4
            for n in (0, 1):
                ceng, csem = ("scalar", "act") if n == 0 else ("vector", "dve")
                wait(ceng, "pe", d_stop[(tq, n)])
                op = (tq // 2) % 2
                wait(ceng, f"out{op}", out_war[op])
                if n == 0:
                    emit("scalar", lambda e, s4=sl4, bk=fbank[(tq, 0)]:
                         nc.scalar.copy(ob_sb[:, s4, 0, :], bk), inc="act")
                else:
                    emit("vector", lambda e, s4=sl4, bk=fbank[(tq, 1)]:
                         nc.vector.tensor_copy(ob_sb[:, s4, 1, :], bk), inc="dve")
                ob_war[(sl4, n)] = (csem, cnt[csem])
            if tq % 2 == 1:
                op = (tq // 2) % 2
                for s4 in (sl4 - 1, sl4):
                    for n in (0, 1):
                        s2, v2 = ob_war[(s4, n)]
                        wait("sync", s2, v2)
                wait("sync", f"out{op}", out_war[op])
                src = ob_sb[:, sl4 - 1:sl4 + 1, :, :]
                dsl = out[(tq - 1) * 128:(tq + 1) * 128, :]
                dst = bass.AP(tensor=dsl.tensor, offset=dsl.offset,
                              ap=[[D, 128], [128 * D, 2], [1, D]])
                emit("sync",
                     lambda e, s3=src, d3=dst: e.dma_start(out=d3, in_=s3),
                     inc=f"out{op}", inc_by=16)
                out_war[op] = cnt[f"out{op}"]

        # ---- final drains ----
        wait("sync", "out0", cnt["out0"])
        wait("sync", "out1", cnt["out1"])
        wait("sync", "bc", cnt["bc"])
        flush_waits()

        # ---------- emit per-engine programs ----------
        def runner(name):
            def _run(eng):
                for e_name, fn in sched:
                    if e_name == name:
                        fn(eng)
            return _run

        block.tensor(runner("tensor"))
        block.scalar(runner("scalar"))
        block.vector(runner("vector"))
        block.sync(runner("sync"))

    return nc


_NC_CACHE = {}
_RUN_KWARGS = {}
_LAST_RESULT = None


def _get_nc(causal: bool):
    if causal not in _NC_CACHE:
        _NC_CACHE[causal] = _build_nc(causal)
    return _NC_CACHE[causal]


def _host_inputs(x, Wqkv, Wout, cos, sin):
    import ml_dtypes
    bf16 = ml_dtypes.bfloat16
    c = np.ascontiguousarray(cos.T)          # [32, T]
    s = np.ascontiguousarray(sin.T)
    cos2 = np.tile(c, (4, 1)).astype(bf16)   # [128, T]
    sinF = np.concatenate([s, -s, s, -s], axis=0).astype(bf16)
    cs = np.concatenate([cos2, sinF], axis=1)  # [128, 2T]
    dm1 = (np.arange(128)[:, None] <= np.arange(128)[None, :]).astype(bf16)
    Wq, Wk, Wv = Wqkv[:, 0:D], Wqkv[:, D:2 * D], Wqkv[:, 2 * D:3 * D]
    in_maps = []
    for core in range(8):
        b, g = divmod(core, NG)
        hs = slice(g * HPC * DH, (g + 1) * HPC * DH)
        in_maps.append({
            "xT": np.ascontiguousarray(x[b].T).astype(bf16),
            "wqk": np.concatenate([Wq[:, hs], Wk[:, hs]], axis=1).astype(bf16),
            "wv": np.ascontiguousarray(Wv[:, hs]).astype(bf16),
            "wo": np.ascontiguousarray(Wout[hs, :]).astype(bf16),
            "cs": cs,
            "dm1": dm1,
        })
    return in_maps


def kernel(x, Wqkv, Wout, cos, sin, mask):
    import sys
    if "/opt/trn_rl_repo" not in sys.path:
        sys.path.insert(0, "/opt/trn_rl_repo")
    from concourse.bass_utils import run_bass_kernel_spmd

    x = np.asarray(x)
    mask = np.asarray(mask)
    m2 = mask.reshape(T, T)
    causal = bool(np.array_equal(m2, np.tril(np.ones((T, T), dtype=bool))))
    if not causal:
        assert m2.all(), "only causal or all-ones masks supported"

    in_maps = _host_inputs(x, np.asarray(Wqkv), np.asarray(Wout),
                           np.asarray(cos), np.asarray(sin))
    nc = _get_nc(causal)
    res = run_bass_kernel_spmd(nc, in_maps, list(range(8)), **_RUN_KWARGS)
    global _LAST_RESULT
    _LAST_RESULT = res
    outs = [np.asarray(r["out"], dtype=np.float32) for r in res.results]
    return np.stack([outs[0] + outs[1] + outs[2] + outs[3],
                     outs[4] + outs[5] + outs[6] + outs[7]])
